# revision 23
# baseline (speedup 1.0000x reference)
"""Trainium2 Bass kernel for DeiT-style attention + depthwise-conv block.

Computes, for x [N=32, L=577, C=768]:
  qkv = x @ w_qkv.T -> q,k,v (12 heads, hd=64)
  attn = softmax(q k^T / 8) @ v
  out  = attn (+ depthwise3x3(v) on patch tokens) @ w_proj.T + b_proj

Sharding: data-parallel over batch, 4 samples per core x 8 NeuronCores.

Structure (per core): the attention inner loop is ACT(exp)-paced, so PE
work from the NEXT sample (qkv/v65 matmuls) and the TWO-samples-back
projection is interleaved as filler between each head's scores and PV
matmuls.  Head pairs write scores into ONE shared [128, 2L] PSUM tile
(disjoint PE row groups run concurrently; disjoint PSUM banks) so exp
runs as a single ACT op per chunk-pair, halving ACT instruction count.
Weights are SBUF-resident.  Softmax normalize = ACT sums-copy -> DVE
fast reciprocal -> gpsimd partition-broadcast -> DVE multiply
(reciprocal_approx_fast requires a base-partition-0 SBUF input on
hardware).  The depthwise conv runs on the DVE as 9 fused
scalar_tensor_tensor taps over flat dense windows of two zero-bordered
pad buffers (the second shifted by one element so odd-offset taps stay
4B-aligned for the 2x DVE mode).
"""
import sys

sys.path.insert(0, "/opt/trn_rl_repo")

import numpy as np

import concourse.bacc as bacc
import concourse.mybir as mybir
import concourse.tile as tile
from concourse.bass_utils import run_bass_kernel_spmd

F32 = mybir.dt.float32
F32R = mybir.dt.float32r
BF16 = mybir.dt.bfloat16
Exp = mybir.ActivationFunctionType.Exp
MULT = mybir.AluOpType.mult
ADD = mybir.AluOpType.add
DIV = mybir.AluOpType.divide

N_CORES = 8
S = 4            # samples per core
C, L, H, HD = 768, 577, 12, 64
CT = C // 128    # 6 channel tiles
KT = 3 * C // 128  # 18 qkv row tiles
SCALE = HD ** -0.5
L_CHUNKS = [(i * 128, min(128, L - i * 128)) for i in range((L + 127) // 128)]
NCH = len(L_CHUNKS)  # 5
IMG = 24         # spatial side; L-1 == IMG*IMG
PAD = IMG + 2    # padded side
PP = PAD * PAD   # 676
CW = PP - 2 * PAD - 2  # 622: flat conv window length
# tap offsets into the flat padded buffer: delta = PAD*dy + dx
EVEN_TAPS = [(0, 0), (2, 2), (26, 3), (28, 5), (52, 6), (54, 8)]  # (delta, w-idx)
ODD_TAPS = [(1, 1), (27, 4), (53, 7)]  # read from the 1-shifted buffer at delta-1

CONV_PE = False  # True: depthwise conv via diag-matmuls on TensorE

# eviction split knobs: 1-in-N PSUM->SBUF evictions go to ACT
EV_ACT_FILL = 2
EV_ACT_ATTN = 6

_CACHE = {}
last_results = None  # BassKernelResults of the most recent run (for test harness)

def _build_nc(repeat=1, stages="full"):
    key = (repeat, stages)
    if key in _CACHE:
        return _CACHE[key]
    nc = bacc.Bacc("TRN2", target_bir_lowering=False, debug=False,
                   num_devices=N_CORES)
    xT_d = nc.declare_dram_parameter("xT", [S, C, L], BF16, isOutput=False)
    wqkvT_d = nc.declare_dram_parameter("wqkvT", [C, 3 * C], BF16, isOutput=False)
    wprojT_d = nc.declare_dram_parameter("wprojT", [C, C], BF16, isOutput=False)
    wdwc_d = nc.declare_dram_parameter("wdwc", [C, 9], F32, isOutput=False)
    bdwc_d = nc.declare_dram_parameter("bdwc", [C, 1], F32, isOutput=False)
    bproj_d = nc.declare_dram_parameter("bproj", [1, C], F32, isOutput=False)
    y_d = nc.declare_dram_parameter("y", [S, L, C], F32, isOutput=True)

    with tile.TileContext(nc) as tc:
        with tc.tile_pool(name="wpool", bufs=1) as wpool, \
             tc.tile_pool(name="work", bufs=2) as work, \
             tc.tile_pool(name="mm", bufs=1, space="PSUM") as psum_mm, \
             tc.tile_pool(name="sc", bufs=2, space="PSUM") as psum_sc, \
             tc.tile_pool(name="pv", bufs=1, space="PSUM") as psum_pv:

            # ---- resident weights (loaded once; q parts first so the
            # first qkv matmuls can start before the rest arrives) ----
            wqkv = []
            for k in range(CT):
                t = wpool.tile([128, 3 * C], BF16, tag="wqkv", bufs=CT,
                               name=f"wqkv{k}")
                wqkv.append(t)
            # single-shot build: prefetch sample-0 x ahead of the weight
            # loads so the first qkv matmuls can start as early as possible
            pre_x0 = None
            if repeat == 1:
                pre_x0 = []
                for k in range(CT):
                    t = work.tile([128, L], BF16, tag="xT", bufs=2 * CT,
                                  name=f"xT0p{k}")
                    nc.sync.dma_start(t[:], xT_d[0, k * 128:(k + 1) * 128, :])
                    pre_x0.append(t)

            wdma = nc.sync
            for part in range(3):
                for k in range(CT):
                    wdma.dma_start(
                        wqkv[k][:, part * C:(part + 1) * C],
                        wqkvT_d[k * 128:(k + 1) * 128, part * C:(part + 1) * C])
            wprojT = []
            for k in range(CT):
                t = wpool.tile([128, C], BF16, tag="wprojT", bufs=CT,
                               name=f"wprojT{k}")
                wdma.dma_start(t[:], wprojT_d[k * 128:(k + 1) * 128, :])
                wprojT.append(t)
            wdwc = []
            bdwc = []
            for k in range(CT):
                t = wpool.tile([128, 9], F32, tag="wdwc", bufs=CT, name=f"wdwc{k}")
                wdma.dma_start(t[:], wdwc_d[k * 128:(k + 1) * 128, :])
                wdwc.append(t)
                t = wpool.tile([128, 1], F32, tag="bdwc", bufs=CT, name=f"bdwc{k}")
                wdma.dma_start(t[:], bdwc_d[k * 128:(k + 1) * 128, :])
                bdwc.append(t)
            bproj_row = wpool.tile([1, C], F32, tag="bprow")
            wdma.dma_start(bproj_row[:], bproj_d[:])
            bproj_bc = wpool.tile([128, C], F32, tag="bpbc")
            nc.gpsimd.partition_broadcast(bproj_bc[:], bproj_row[:])

            # persistent zero-bordered conv pad buffers (interior rewritten
            # per use; borders stay zero).  vpadB holds the same image
            # shifted left by one flat element so odd tap offsets become
            # even (4B-aligned) reads.
            vpad, vpadB = [], []
            for i in range(2):
                t = wpool.tile([128, PP], BF16, tag="vpad", bufs=2,
                               name=f"vpad{i}")
                nc.vector.memset(t[:], 0.0)
                vpad.append(t)
                t = wpool.tile([128, PP], BF16, tag="vpadB", bufs=2,
                               name=f"vpadB{i}")
                nc.vector.memset(t[:], 0.0)
                vpadB.append(t)
            # persistent v65 tiles (two sets of NCH); ones column written once
            v65_all = []
            for i in range(2 * NCH):
                t = wpool.tile([128, H * 65], BF16, tag="v65", bufs=2 * NCH,
                               name=f"v65_{i}")
                t3 = t[:].rearrange("p (h w) -> p h w", h=H, w=65)
                nc.vector.memset(t3[:, :, 64:65], 1.0)
                v65_all.append(t)

            import contextlib
            rep_ctx = tc.For_i(0, repeat, 1) if repeat > 1 else contextlib.nullcontext()
            with rep_ctx:
                state = {}
                evict_ctr = [0]
                evict_act_period = [EV_ACT_FILL]  # 1-in-N evictions go to ACT

                def evict(dst_ap, src_ap):
                    # split PSUM->SBUF evictions between ACT and DVE
                    if evict_ctr[0] % evict_act_period[0] == 0:
                        nc.scalar.copy(dst_ap, src_ap)
                    else:
                        nc.vector.tensor_copy(dst_ap, src_ap)
                    evict_ctr[0] += 1

                mm_ctr = [0]
                mm_alt = [False]  # when True, alternate mm/sc pools

                def mm_tile():
                    mm_ctr[0] += 1
                    if mm_alt[0] and mm_ctr[0] % 2 == 0:
                        return psum_sc.tile([128, 768], F32, tag="sc",
                                            name="mmsc")
                    return psum_mm.tile([128, 768], F32, tag="mm", name="mmp")

                def emit_sample_inputs(s):
                    st = {"xT": [], "qk": [], "vch": [],
                          "v65": [v65_all[(s % 2) * NCH + ci] for ci in range(NCH)]}
                    if s == 0 and pre_x0 is not None:
                        st["xT"] = pre_x0
                        state[s] = st
                        return st
                    for k in range(CT):
                        t = work.tile([128, L], BF16, tag="xT", bufs=2 * CT,
                                      name=f"xT{k}")
                        nc.sync.dma_start(t[:], xT_d[s, k * 128:(k + 1) * 128, :])
                        st["xT"].append(t)
                    state[s] = st
                    return st

                def qkv_mtile(s, m):
                    st = state[s]
                    p = mm_tile()
                    for k in range(CT):
                        w_ap = wqkv[k][:, m * 128:(m + 1) * 128]
                        for (n0, nn) in ((0, 512), (512, 65)):
                            nc.tensor.matmul(
                                p[:, n0:n0 + nn], w_ap,
                                st["xT"][k][:, n0:n0 + nn],
                                start=(k == 0), stop=(k == CT - 1))
                    if m < 12:
                        dst = work.tile([128, L], BF16, tag="qk", bufs=24,
                                        name=f"qkv{m}")
                        evict(dst[:], p[:, 0:L])
                        st["qk"].append(dst)
                    else:
                        # vch padded to 640 so the tail v65 transpose can
                        # read a full 128-wide window (xbar constraint)
                        dst = work.tile([128, 640], BF16, tag="vch",
                                        bufs=2 * CT, name=f"qkv{m}")
                        evict(dst[:, 0:L], p[:, 0:L])
                        nc.vector.memset(dst[:, L:640], 0.0)
                        st["vch"].append(dst)

                def v65_chunk(s, ci):
                    # v65 [l, c] from vch [c, l] by xbar DMA transpose into a
                    # 128B-aligned temp (misaligned 65-stride dests corrupt
                    # on hardware), then one DVE copy into the [v|1] layout;
                    # frees the TensorE of the duplicate v matmuls.  The tail
                    # chunk reads the zero-padded 512:640 window; its junk
                    # rows (>= lp) are never read by PV.
                    st = state[s]
                    (l0, lp) = L_CHUNKS[ci]
                    w0 = min(l0, 640 - 128)
                    t = st["v65"][ci]
                    tmp = work.tile([128, 768], BF16, tag="v65t", bufs=2,
                                    name="v65t")
                    for cj in range(CT):
                        eng = nc.sync if (cj % 2) else nc.scalar
                        eng.dma_start_transpose(
                            tmp[0:128, cj * 128:(cj + 1) * 128],
                            st["vch"][cj][0:128, w0:w0 + 128])
                    nc.vector.tensor_copy(
                        t[0:lp].rearrange("p (h w) -> p h w", h=H, w=65)[:, :, 0:64],
                        tmp[0:lp].rearrange("p (h w) -> p h w", h=H, w=64))

                def make_fill_thunks(s):
                    # vch m-tiles (12..17) first so the v65 transposes can
                    # start as early as possible
                    return ([lambda m=m: qkv_mtile(s, m)
                             for m in list(range(12, KT)) + list(range(12))]
                            + [lambda ci=ci: v65_chunk(s, ci) for ci in range(NCH)])

                def proj_chunk(s, ci):
                    st = state[s]
                    (l0, lp) = L_CHUNKS[ci]
                    attn = st["attn"]
                    p = mm_tile()
                    for (n0, nn) in ((0, 512), (512, 256)):
                        for k in range(CT):
                            nc.tensor.matmul(
                                p[0:lp, n0:n0 + nn],
                                attn[k][:, l0:l0 + lp],
                                wprojT[k][:, n0:n0 + nn],
                                start=(k == 0), stop=(k == CT - 1))
                    ysb = work.tile([128, C], F32, tag="ysb", bufs=2)
                    nc.vector.tensor_tensor(
                        out=ysb[0:lp, :], in0=p[0:lp, :], in1=bproj_bc[0:lp, :],
                        op=ADD)
                    nc.sync.dma_start(y_d[s, l0:l0 + lp, :], ysb[0:lp, :])

                def make_proj_thunks(s):
                    return [lambda ci=ci: proj_chunk(s, ci) for ci in range(NCH)]

                def scores_pair(s, hp):
                    # heads 2hp (rows 0:64) and 2hp+1 (rows 64:128) emitted
                    # chunk-interleaved: adjacent matmuls hit disjoint PE row
                    # groups AND disjoint PSUM banks (separate sc-ring slots),
                    # so they run concurrently on hardware
                    st = state[s]
                    qt = st["qk"][hp]
                    kt_ = st["qk"][6 + hp]
                    expA, expB = [], []
                    order = [(l0, lp, hb, e) for (l0, lp) in L_CHUNKS
                             for hb, e in ((0, expA), (64, expB))]
                    for (l0, lp, hb, exps) in order:
                        p = psum_sc.tile([128, 768], F32, tag="sc",
                                         name="scp")
                        for (n0, nn) in ((0, 512), (512, 65)):
                            nc.tensor.matmul(p[0:lp, n0:n0 + nn],
                                             kt_[hb:hb + 64, l0:l0 + lp],
                                             qt[hb:hb + 64, n0:n0 + nn],
                                             start=True, stop=True)
                        e = work.tile([128, L], BF16, tag="expS", bufs=12,
                                      name="expSt")
                        nc.scalar.activation(e[0:lp, :], p[0:lp, 0:L], Exp,
                                             scale=SCALE)
                        exps.append(e)
                    return expA, expB

                def pv_head(s, h, exps):
                    st = state[s]
                    pv = psum_pv.tile([128, L], F32, tag="pv")
                    for ci, (l0, lp) in enumerate(L_CHUNKS):
                        for (n0, nn) in ((0, 512), (512, 65)):
                            nc.tensor.matmul(
                                pv[0:65, n0:n0 + nn],
                                st["v65"][ci][0:lp, h * 65:(h + 1) * 65],
                                exps[ci][0:lp, n0:n0 + nn],
                                start=(ci == 0), stop=(ci == NCH - 1))
                    # NOTE: reciprocal_approx_fast needs a base-partition-0
                    # SBUF operand (PSUM or partition-offset reads return
                    # garbage on hardware), so stage the sums row via ACT
                    sums = work.tile([1, L], F32, tag="sums", bufs=2,
                                     name="sums")
                    nc.scalar.copy(sums[:], pv[64:65, :])
                    rec = work.tile([1, L], F32, tag="rec", bufs=2, name="rec")
                    nc.vector.reciprocal_approx_fast(out=rec[:], in_=sums[:])
                    bc = work.tile([64, L], F32, tag="bc", bufs=2, name="bc")
                    nc.gpsimd.partition_broadcast(bc[:], rec[:])
                    hb = (h % 2) * 64
                    nc.vector.tensor_tensor(
                        out=st["attn"][h // 2][hb:hb + 64, :],
                        in0=pv[0:64, :], in1=bc[:], op=MULT)

                def conv_prep(s, ct):
                    st = state[s]
                    vp = vpad[ct % 2]
                    vp3 = vp[:].rearrange("p (y x) -> p y x", y=PAD, x=PAD)
                    src_ = st["vch"][ct][:, 1:L].rearrange("p (y x) -> p y x",
                                                           y=IMG, x=IMG)
                    nc.vector.tensor_copy(vp3[:, 1:1 + IMG, 1:1 + IMG], src_)
                    if CONV_PE:
                        vpB = vpadB[ct % 2]
                        vpB3 = vpB[:].rearrange("p (y x) -> p y x",
                                                y=PAD, x=PAD)
                        nc.vector.tensor_copy(vpB3[:, 1:1 + IMG, 0:IMG], src_)
                        p = psum_sc.tile([128, 768], F32, tag="sc",
                                         name="caccp")
                        taps = ([(d, w, vp) for (d, w) in EVEN_TAPS]
                                + [(d - 1, w, vpB) for (d, w) in ODD_TAPS])
                        for (n0, nn) in ((0, 512), (512, 110)):
                            for i, (dd, w, buf) in enumerate(taps):
                                nc.tensor.matmul(
                                    p[:, n0:n0 + nn],
                                    wdiag[ct][:, w * 128:(w + 1) * 128],
                                    buf[:, dd + n0:dd + n0 + nn],
                                    start=(i == 0), stop=(i == 8))
                        return p
                    acc = work.tile([128, IMG * IMG], BF16, tag="cacc", bufs=2,
                                    name="cacc")
                    acc3 = acc[:].rearrange("p (y x) -> p y x", y=IMG, x=IMG)

                    def tap(t):
                        return vp3[:, t // 3:t // 3 + IMG, t % 3:t % 3 + IMG]

                    nc.vector.tensor_scalar(
                        out=acc3, in0=tap(4), scalar1=wdwc[ct][:, 4:5],
                        scalar2=None, op0=MULT)
                    for t in [0, 1, 2, 3, 5, 6, 7, 8]:
                        tmp = work.tile([128, IMG * IMG], BF16, tag="ctmp",
                                        bufs=6, name="ctmp")
                        tmp3 = tmp[:].rearrange("p (y x) -> p y x", y=IMG, x=IMG)
                        nc.vector.tensor_scalar(
                            out=tmp3, in0=tap(t), scalar1=wdwc[ct][:, t:t + 1],
                            scalar2=None, op0=MULT)
                        nc.vector.tensor_tensor(out=acc[:], in0=acc[:],
                                                in1=tmp[:], op=ADD)
                    return acc

                def conv_add(s, ct, acc):
                    if CONV_PE:
                        acc_int = acc[:, 0:IMG * PAD].rearrange(
                            "p (y x) -> p y x", y=IMG, x=PAD)[:, :, 0:IMG]
                        nc.vector.scalar_tensor_tensor(
                            out=state[s]["attn"][ct][:, 1:L].rearrange(
                                "p (y x) -> p y x", y=IMG, x=IMG),
                            in0=acc_int,
                            scalar=bdwc[ct][:, 0:1],
                            in1=state[s]["attn"][ct][:, 1:L].rearrange(
                                "p (y x) -> p y x", y=IMG, x=IMG),
                            op0=ADD, op1=ADD)
                        return
                    # attn[:, 1:] += acc + b_dwc
                    nc.vector.scalar_tensor_tensor(
                        out=state[s]["attn"][ct][:, 1:L], in0=acc[:],
                        scalar=bdwc[ct][:, 0:1],
                        in1=state[s]["attn"][ct][:, 1:L],
                        op0=ADD, op1=ADD)

                # ---- prologue: sample 0 inputs + qkv/v65 emitted directly
                # (mm/sc pool alternation -- nothing else needs sc yet) ----
                emit_sample_inputs(0)
                mm_alt[0] = True
                for t in make_fill_thunks(0):
                    t()
                mm_alt[0] = False

                for s in range(S):
                    st = state[s]
                    st["attn"] = [work.tile([128, L], BF16, tag="attn", bufs=18,
                                            name=f"attn{ct}") for ct in range(CT)]
                    fillers = []
                    if s + 1 < S:
                        emit_sample_inputs(s + 1)
                        fillers += make_fill_thunks(s + 1)
                    # projections trail by two samples so the tail sample's
                    # exp-waits still have PE filler work
                    if s - 2 >= 0:
                        fillers += make_proj_thunks(s - 2)
                    if s == S - 1:
                        fillers += make_proj_thunks(s - 1)

                    if stages == "qkv":
                        zsrc = work.tile([128, L], F32, tag="zsrc", bufs=1,
                                         name="zsrc")
                        nc.vector.memset(zsrc[:], 0.0)
                        for ct in range(CT):
                            nc.vector.tensor_copy(st["attn"][ct][:], zsrc[:])
                        for t in fillers:
                            t()
                        continue

                    # ACT paces the attention inner loop; keep it mostly exp
                    evict_act_period[0] = EV_ACT_ATTN
                    nf = len(fillers)
                    done = 0
                    for hp in range(H // 2):
                        expA, expB = scores_pair(s, hp)
                        if stages == "full":
                            acc = conv_prep(s, hp)
                        # PE fillers between scores and PV cover the exp wait
                        target = ((2 * hp + 1) * nf) // H
                        while done < target:
                            fillers[done]()
                            done += 1
                        pv_head(s, 2 * hp, expA)
                        target = ((2 * hp + 2) * nf) // H
                        while done < target:
                            fillers[done]()
                            done += 1
                        pv_head(s, 2 * hp + 1, expB)
                        if stages == "full":
                            conv_add(s, hp, acc)
                    evict_act_period[0] = EV_ACT_FILL

                # final projection (no attention loop left to hide it in)
                mm_alt[0] = True
                for t in make_proj_thunks(S - 1):
                    t()
                mm_alt[0] = False

    nc.compile()
    _CACHE[key] = nc
    return nc


def make_in_maps(x, w_qkv, w_proj, b_proj, w_dwc, b_dwc):
    x = np.asarray(x, dtype=np.float32)
    N = x.shape[0]
    assert N == N_CORES * S
    import ml_dtypes
    wqkvT = np.ascontiguousarray(
        np.asarray(w_qkv, np.float32).T.astype(ml_dtypes.bfloat16))    # [C, 3C]
    wprojT = np.ascontiguousarray(
        np.asarray(w_proj, np.float32).T.astype(ml_dtypes.bfloat16))   # [C, C]
    wdwc9 = np.ascontiguousarray(np.asarray(w_dwc, np.float32).reshape(C, 9))
    bdwc = np.ascontiguousarray(np.asarray(b_dwc, np.float32).reshape(C, 1))
    bproj = np.ascontiguousarray(np.asarray(b_proj, np.float32).reshape(1, C))

    in_maps = []
    for i in range(N_CORES):
        xs = x[i * S:(i + 1) * S]                       # [S, L, C]
        xT = np.ascontiguousarray(
            xs.transpose(0, 2, 1).astype(ml_dtypes.bfloat16))  # [S, C, L]
        in_maps.append({"xT": xT, "wqkvT": wqkvT, "wprojT": wprojT,
                        "wdwc": wdwc9, "bdwc": bdwc, "bproj": bproj})
    return in_maps


def kernel(x, w_qkv, w_proj, b_proj, w_dwc, b_dwc):
    global last_results
    nc = _build_nc()
    in_maps = make_in_maps(x, w_qkv, w_proj, b_proj, w_dwc, b_dwc)
    last_results = run_bass_kernel_spmd(nc, in_maps, list(range(N_CORES)))
    y = np.concatenate([r["y"] for r in last_results.results], axis=0)
    return y.astype(np.float32)


# revision 24
# speedup vs baseline: 1.0059x; 1.0059x over previous
"""Trainium2 Bass kernel for DeiT-style attention + depthwise-conv block.

Computes, for x [N=32, L=577, C=768]:
  qkv = x @ w_qkv.T -> q,k,v (12 heads, hd=64)
  attn = softmax(q k^T / 8) @ v
  out  = attn (+ depthwise3x3(v) on patch tokens) @ w_proj.T + b_proj

Sharding: data-parallel over batch, 4 samples per core x 8 NeuronCores.

Structure (per core): the attention inner loop is ACT(exp)-paced, so PE
work from the NEXT sample (qkv/v65 matmuls) and the TWO-samples-back
projection is interleaved as filler between each head's scores and PV
matmuls.  Head pairs write scores into ONE shared [128, 2L] PSUM tile
(disjoint PE row groups run concurrently; disjoint PSUM banks) so exp
runs as a single ACT op per chunk-pair, halving ACT instruction count.
Weights are SBUF-resident.  Softmax normalize = ACT sums-copy -> DVE
fast reciprocal -> gpsimd partition-broadcast -> DVE multiply
(reciprocal_approx_fast requires a base-partition-0 SBUF input on
hardware).  The depthwise conv runs on the DVE as 9 fused
scalar_tensor_tensor taps over flat dense windows of two zero-bordered
pad buffers (the second shifted by one element so odd-offset taps stay
4B-aligned for the 2x DVE mode).
"""
import sys

sys.path.insert(0, "/opt/trn_rl_repo")

import numpy as np

import concourse.bacc as bacc
import concourse.mybir as mybir
import concourse.tile as tile
from concourse.bass_utils import run_bass_kernel_spmd

F32 = mybir.dt.float32
F32R = mybir.dt.float32r
BF16 = mybir.dt.bfloat16
Exp = mybir.ActivationFunctionType.Exp
MULT = mybir.AluOpType.mult
ADD = mybir.AluOpType.add
DIV = mybir.AluOpType.divide

N_CORES = 8
S = 4            # samples per core
C, L, H, HD = 768, 577, 12, 64
CT = C // 128    # 6 channel tiles
KT = 3 * C // 128  # 18 qkv row tiles
SCALE = HD ** -0.5
L_CHUNKS = [(i * 128, min(128, L - i * 128)) for i in range((L + 127) // 128)]
NCH = len(L_CHUNKS)  # 5
IMG = 24         # spatial side; L-1 == IMG*IMG
PAD = IMG + 2    # padded side
PP = PAD * PAD   # 676
CW = PP - 2 * PAD - 2  # 622: flat conv window length
# tap offsets into the flat padded buffer: delta = PAD*dy + dx
EVEN_TAPS = [(0, 0), (2, 2), (26, 3), (28, 5), (52, 6), (54, 8)]  # (delta, w-idx)
ODD_TAPS = [(1, 1), (27, 4), (53, 7)]  # read from the 1-shifted buffer at delta-1

CONV_PE = False  # True: depthwise conv via diag-matmuls on TensorE

# eviction split knobs: 1-in-N PSUM->SBUF evictions go to ACT
EV_ACT_FILL = 2
EV_ACT_ATTN = 12

_CACHE = {}
last_results = None  # BassKernelResults of the most recent run (for test harness)

def _build_nc(repeat=1, stages="full"):
    key = (repeat, stages)
    if key in _CACHE:
        return _CACHE[key]
    nc = bacc.Bacc("TRN2", target_bir_lowering=False, debug=False,
                   num_devices=N_CORES)
    xT_d = nc.declare_dram_parameter("xT", [S, C, L], BF16, isOutput=False)
    wqkvT_d = nc.declare_dram_parameter("wqkvT", [C, 3 * C], BF16, isOutput=False)
    wprojT_d = nc.declare_dram_parameter("wprojT", [C, C], BF16, isOutput=False)
    wdwc_d = nc.declare_dram_parameter("wdwc", [C, 9], F32, isOutput=False)
    bdwc_d = nc.declare_dram_parameter("bdwc", [C, 1], F32, isOutput=False)
    bproj_d = nc.declare_dram_parameter("bproj", [1, C], F32, isOutput=False)
    y_d = nc.declare_dram_parameter("y", [S, L, C], F32, isOutput=True)

    with tile.TileContext(nc) as tc:
        with tc.tile_pool(name="wpool", bufs=1) as wpool, \
             tc.tile_pool(name="work", bufs=2) as work, \
             tc.tile_pool(name="mm", bufs=1, space="PSUM") as psum_mm, \
             tc.tile_pool(name="sc", bufs=2, space="PSUM") as psum_sc, \
             tc.tile_pool(name="pv", bufs=1, space="PSUM") as psum_pv:

            # ---- resident weights (loaded once; q parts first so the
            # first qkv matmuls can start before the rest arrives) ----
            wqkv = []
            for k in range(CT):
                t = wpool.tile([128, 3 * C], BF16, tag="wqkv", bufs=CT,
                               name=f"wqkv{k}")
                wqkv.append(t)
            # single-shot build: prefetch sample-0 x ahead of the weight
            # loads so the first qkv matmuls can start as early as possible
            pre_x0 = None
            if repeat == 1:
                pre_x0 = []
                for k in range(CT):
                    t = work.tile([128, L], BF16, tag="xT", bufs=2 * CT,
                                  name=f"xT0p{k}")
                    nc.sync.dma_start(t[:], xT_d[0, k * 128:(k + 1) * 128, :])
                    pre_x0.append(t)

            wdma = nc.sync
            for part in range(3):
                for k in range(CT):
                    wdma.dma_start(
                        wqkv[k][:, part * C:(part + 1) * C],
                        wqkvT_d[k * 128:(k + 1) * 128, part * C:(part + 1) * C])
            wprojT = []
            for k in range(CT):
                t = wpool.tile([128, C], BF16, tag="wprojT", bufs=CT,
                               name=f"wprojT{k}")
                wdma.dma_start(t[:], wprojT_d[k * 128:(k + 1) * 128, :])
                wprojT.append(t)
            wdwc = []
            bdwc = []
            for k in range(CT):
                t = wpool.tile([128, 9], F32, tag="wdwc", bufs=CT, name=f"wdwc{k}")
                wdma.dma_start(t[:], wdwc_d[k * 128:(k + 1) * 128, :])
                wdwc.append(t)
                t = wpool.tile([128, 1], F32, tag="bdwc", bufs=CT, name=f"bdwc{k}")
                wdma.dma_start(t[:], bdwc_d[k * 128:(k + 1) * 128, :])
                bdwc.append(t)
            bproj_row = wpool.tile([1, C], F32, tag="bprow")
            wdma.dma_start(bproj_row[:], bproj_d[:])
            bproj_bc = wpool.tile([128, C], F32, tag="bpbc")
            nc.gpsimd.partition_broadcast(bproj_bc[:], bproj_row[:])

            # persistent zero-bordered conv pad buffers (interior rewritten
            # per use; borders stay zero).  vpadB holds the same image
            # shifted left by one flat element so odd tap offsets become
            # even (4B-aligned) reads.
            vpad = []
            for i in range(2):
                t = wpool.tile([128, PP], BF16, tag="vpad", bufs=2,
                               name=f"vpad{i}")
                nc.vector.memset(t[:], 0.0)
                vpad.append(t)
            # persistent v65 tiles (two sets of NCH); ones column written once
            v65_all = []
            for i in range(2 * NCH):
                t = wpool.tile([128, H * 65], BF16, tag="v65", bufs=2 * NCH,
                               name=f"v65_{i}")
                t3 = t[:].rearrange("p (h w) -> p h w", h=H, w=65)
                nc.vector.memset(t3[:, :, 64:65], 1.0)
                v65_all.append(t)

            import contextlib
            rep_ctx = tc.For_i(0, repeat, 1) if repeat > 1 else contextlib.nullcontext()
            with rep_ctx:
                state = {}
                evict_ctr = [0]
                evict_act_period = [EV_ACT_FILL]  # 1-in-N evictions go to ACT

                def evict(dst_ap, src_ap):
                    # split PSUM->SBUF evictions between ACT and DVE
                    if evict_ctr[0] % evict_act_period[0] == 0:
                        nc.scalar.copy(dst_ap, src_ap)
                    else:
                        nc.vector.tensor_copy(dst_ap, src_ap)
                    evict_ctr[0] += 1

                mm_ctr = [0]
                mm_alt = [False]  # when True, alternate mm/sc pools

                def mm_tile():
                    mm_ctr[0] += 1
                    if mm_alt[0] and mm_ctr[0] % 2 == 0:
                        return psum_sc.tile([128, 768], F32, tag="sc",
                                            name="mmsc")
                    return psum_mm.tile([128, 768], F32, tag="mm", name="mmp")

                def emit_sample_inputs(s):
                    st = {"xT": [], "qk": [], "vch": [],
                          "v65": [v65_all[(s % 2) * NCH + ci] for ci in range(NCH)]}
                    if s == 0 and pre_x0 is not None:
                        st["xT"] = pre_x0
                        state[s] = st
                        return st
                    for k in range(CT):
                        t = work.tile([128, L], BF16, tag="xT", bufs=2 * CT,
                                      name=f"xT{k}")
                        nc.sync.dma_start(t[:], xT_d[s, k * 128:(k + 1) * 128, :])
                        st["xT"].append(t)
                    state[s] = st
                    return st

                def qkv_mtile(s, m):
                    st = state[s]
                    p = mm_tile()
                    for k in range(CT):
                        w_ap = wqkv[k][:, m * 128:(m + 1) * 128]
                        for (n0, nn) in ((0, 512), (512, 65)):
                            nc.tensor.matmul(
                                p[:, n0:n0 + nn], w_ap,
                                st["xT"][k][:, n0:n0 + nn],
                                start=(k == 0), stop=(k == CT - 1))
                    if m < 12:
                        dst = work.tile([128, L], BF16, tag="qk", bufs=24,
                                        name=f"qkv{m}")
                        evict(dst[:], p[:, 0:L])
                        st["qk"].append(dst)
                    else:
                        # vch padded to 640 so the tail v65 transpose can
                        # read a full 128-wide window (xbar constraint)
                        dst = work.tile([128, 640], BF16, tag="vch",
                                        bufs=2 * CT, name=f"qkv{m}")
                        evict(dst[:, 0:L], p[:, 0:L])
                        nc.vector.memset(dst[:, L:640], 0.0)
                        st["vch"].append(dst)

                def v65_chunk(s, ci):
                    # v65 [l, c] from vch [c, l] by xbar DMA transpose into a
                    # 128B-aligned temp (misaligned 65-stride dests corrupt
                    # on hardware), then one DVE copy into the [v|1] layout;
                    # frees the TensorE of the duplicate v matmuls.  The tail
                    # chunk reads the zero-padded 512:640 window; its junk
                    # rows (>= lp) are never read by PV.
                    st = state[s]
                    (l0, lp) = L_CHUNKS[ci]
                    w0 = min(l0, 640 - 128)
                    t = st["v65"][ci]
                    tmp = work.tile([128, 768], BF16, tag="v65t", bufs=2,
                                    name="v65t")
                    for cj in range(CT):
                        eng = nc.sync if (cj % 2) else nc.scalar
                        eng.dma_start_transpose(
                            tmp[0:128, cj * 128:(cj + 1) * 128],
                            st["vch"][cj][0:128, w0:w0 + 128])
                    nc.vector.tensor_copy(
                        t[0:lp].rearrange("p (h w) -> p h w", h=H, w=65)[:, :, 0:64],
                        tmp[0:lp].rearrange("p (h w) -> p h w", h=H, w=64))

                def make_fill_thunks(s):
                    # vch m-tiles (12..17) first so the v65 transposes can
                    # start as early as possible
                    return ([lambda m=m: qkv_mtile(s, m)
                             for m in list(range(12, KT)) + list(range(12))]
                            + [lambda ci=ci: v65_chunk(s, ci) for ci in range(NCH)])

                def proj_chunk(s, ci):
                    st = state[s]
                    (l0, lp) = L_CHUNKS[ci]
                    attn = st["attn"]
                    p = mm_tile()
                    for (n0, nn) in ((0, 512), (512, 256)):
                        for k in range(CT):
                            nc.tensor.matmul(
                                p[0:lp, n0:n0 + nn],
                                attn[k][:, l0:l0 + lp],
                                wprojT[k][:, n0:n0 + nn],
                                start=(k == 0), stop=(k == CT - 1))
                    ysb = work.tile([128, C], F32, tag="ysb", bufs=2)
                    nc.vector.tensor_tensor(
                        out=ysb[0:lp, :], in0=p[0:lp, :], in1=bproj_bc[0:lp, :],
                        op=ADD)
                    nc.sync.dma_start(y_d[s, l0:l0 + lp, :], ysb[0:lp, :])

                def make_proj_thunks(s):
                    return [lambda ci=ci: proj_chunk(s, ci) for ci in range(NCH)]

                def scores_pair(s, hp):
                    # heads 2hp (rows 0:64) and 2hp+1 (rows 64:128) emitted
                    # chunk-interleaved: adjacent matmuls hit disjoint PE row
                    # groups AND disjoint PSUM banks (separate sc-ring slots),
                    # so they run concurrently on hardware
                    st = state[s]
                    qt = st["qk"][hp]
                    kt_ = st["qk"][6 + hp]
                    expA, expB = [], []
                    order = [(l0, lp, hb, e) for (l0, lp) in L_CHUNKS
                             for hb, e in ((0, expA), (64, expB))]
                    for (l0, lp, hb, exps) in order:
                        p = psum_sc.tile([128, 768], F32, tag="sc",
                                         name="scp")
                        for (n0, nn) in ((0, 512), (512, 65)):
                            nc.tensor.matmul(p[0:lp, n0:n0 + nn],
                                             kt_[hb:hb + 64, l0:l0 + lp],
                                             qt[hb:hb + 64, n0:n0 + nn],
                                             start=True, stop=True)
                        e = work.tile([128, L], BF16, tag="expS", bufs=12,
                                      name="expSt")
                        nc.scalar.activation(e[0:lp, :], p[0:lp, 0:L], Exp,
                                             scale=SCALE)
                        exps.append(e)
                    return expA, expB

                def pv_head(s, h, exps):
                    st = state[s]
                    pv = psum_pv.tile([128, L], F32, tag="pv")
                    for ci, (l0, lp) in enumerate(L_CHUNKS):
                        for (n0, nn) in ((0, 512), (512, 65)):
                            nc.tensor.matmul(
                                pv[0:65, n0:n0 + nn],
                                st["v65"][ci][0:lp, h * 65:(h + 1) * 65],
                                exps[ci][0:lp, n0:n0 + nn],
                                start=(ci == 0), stop=(ci == NCH - 1))
                    # NOTE: reciprocal_approx_fast needs a base-partition-0
                    # SBUF operand (PSUM or partition-offset reads return
                    # garbage on hardware), so stage the sums row via ACT
                    sums = work.tile([1, L], F32, tag="sums", bufs=2,
                                     name="sums")
                    nc.scalar.copy(sums[:], pv[64:65, :])
                    rec = work.tile([1, L], F32, tag="rec", bufs=2, name="rec")
                    nc.vector.reciprocal_approx_fast(out=rec[:], in_=sums[:])
                    bc = work.tile([64, L], F32, tag="bc", bufs=2, name="bc")
                    nc.gpsimd.partition_broadcast(bc[:], rec[:])
                    hb = (h % 2) * 64
                    nc.vector.tensor_tensor(
                        out=st["attn"][h // 2][hb:hb + 64, :],
                        in0=pv[0:64, :], in1=bc[:], op=MULT)

                def conv_prep(s, ct):
                    st = state[s]
                    vp = vpad[ct % 2]
                    vp3 = vp[:].rearrange("p (y x) -> p y x", y=PAD, x=PAD)
                    src_ = st["vch"][ct][:, 1:L].rearrange("p (y x) -> p y x",
                                                           y=IMG, x=IMG)
                    nc.vector.tensor_copy(vp3[:, 1:1 + IMG, 1:1 + IMG], src_)
                    if CONV_PE:
                        vpB = vpadB[ct % 2]
                        vpB3 = vpB[:].rearrange("p (y x) -> p y x",
                                                y=PAD, x=PAD)
                        nc.vector.tensor_copy(vpB3[:, 1:1 + IMG, 0:IMG], src_)
                        p = psum_sc.tile([128, 768], F32, tag="sc",
                                         name="caccp")
                        taps = ([(d, w, vp) for (d, w) in EVEN_TAPS]
                                + [(d - 1, w, vpB) for (d, w) in ODD_TAPS])
                        for (n0, nn) in ((0, 512), (512, 110)):
                            for i, (dd, w, buf) in enumerate(taps):
                                nc.tensor.matmul(
                                    p[:, n0:n0 + nn],
                                    wdiag[ct][:, w * 128:(w + 1) * 128],
                                    buf[:, dd + n0:dd + n0 + nn],
                                    start=(i == 0), stop=(i == 8))
                        return p
                    acc = work.tile([128, IMG * IMG], BF16, tag="cacc", bufs=2,
                                    name="cacc")
                    acc3 = acc[:].rearrange("p (y x) -> p y x", y=IMG, x=IMG)

                    def tap(t):
                        return vp3[:, t // 3:t // 3 + IMG, t % 3:t % 3 + IMG]

                    nc.vector.tensor_scalar(
                        out=acc3, in0=tap(4), scalar1=wdwc[ct][:, 4:5],
                        scalar2=None, op0=MULT)
                    for t in [0, 1, 2, 3, 5, 6, 7, 8]:
                        tmp = work.tile([128, IMG * IMG], BF16, tag="ctmp",
                                        bufs=6, name="ctmp")
                        tmp3 = tmp[:].rearrange("p (y x) -> p y x", y=IMG, x=IMG)
                        nc.vector.tensor_scalar(
                            out=tmp3, in0=tap(t), scalar1=wdwc[ct][:, t:t + 1],
                            scalar2=None, op0=MULT)
                        nc.vector.tensor_tensor(out=acc[:], in0=acc[:],
                                                in1=tmp[:], op=ADD)
                    return acc

                def conv_add(s, ct, acc):
                    if CONV_PE:
                        acc_int = acc[:, 0:IMG * PAD].rearrange(
                            "p (y x) -> p y x", y=IMG, x=PAD)[:, :, 0:IMG]
                        nc.vector.scalar_tensor_tensor(
                            out=state[s]["attn"][ct][:, 1:L].rearrange(
                                "p (y x) -> p y x", y=IMG, x=IMG),
                            in0=acc_int,
                            scalar=bdwc[ct][:, 0:1],
                            in1=state[s]["attn"][ct][:, 1:L].rearrange(
                                "p (y x) -> p y x", y=IMG, x=IMG),
                            op0=ADD, op1=ADD)
                        return
                    # attn[:, 1:] += acc + b_dwc
                    nc.vector.scalar_tensor_tensor(
                        out=state[s]["attn"][ct][:, 1:L], in0=acc[:],
                        scalar=bdwc[ct][:, 0:1],
                        in1=state[s]["attn"][ct][:, 1:L],
                        op0=ADD, op1=ADD)

                # ---- prologue: sample 0 inputs + qkv/v65 emitted directly
                # (mm/sc pool alternation -- nothing else needs sc yet) ----
                emit_sample_inputs(0)
                mm_alt[0] = True
                for t in make_fill_thunks(0):
                    t()
                mm_alt[0] = False

                for s in range(S):
                    st = state[s]
                    st["attn"] = [work.tile([128, L], BF16, tag="attn", bufs=18,
                                            name=f"attn{ct}") for ct in range(CT)]
                    fillers = []
                    if s + 1 < S:
                        emit_sample_inputs(s + 1)
                        fillers += make_fill_thunks(s + 1)
                    # projections trail by two samples so the tail sample's
                    # exp-waits still have PE filler work
                    if s - 2 >= 0:
                        fillers += make_proj_thunks(s - 2)
                    if s == S - 1:
                        fillers += make_proj_thunks(s - 1)

                    if stages == "qkv":
                        zsrc = work.tile([128, L], F32, tag="zsrc", bufs=1,
                                         name="zsrc")
                        nc.vector.memset(zsrc[:], 0.0)
                        for ct in range(CT):
                            nc.vector.tensor_copy(st["attn"][ct][:], zsrc[:])
                        for t in fillers:
                            t()
                        continue

                    # ACT paces the attention inner loop; keep it mostly exp
                    evict_act_period[0] = EV_ACT_ATTN
                    nf = len(fillers)
                    done = 0
                    for hp in range(H // 2):
                        expA, expB = scores_pair(s, hp)
                        if stages == "full":
                            acc = conv_prep(s, hp)
                        # PE fillers between scores and PV cover the exp wait
                        target = ((2 * hp + 1) * nf) // H
                        while done < target:
                            fillers[done]()
                            done += 1
                        pv_head(s, 2 * hp, expA)
                        target = ((2 * hp + 2) * nf) // H
                        while done < target:
                            fillers[done]()
                            done += 1
                        pv_head(s, 2 * hp + 1, expB)
                        if stages == "full":
                            conv_add(s, hp, acc)
                    evict_act_period[0] = EV_ACT_FILL

                # final projection (no attention loop left to hide it in)
                mm_alt[0] = True
                for t in make_proj_thunks(S - 1):
                    t()
                mm_alt[0] = False

    nc.compile()
    _CACHE[key] = nc
    return nc


def make_in_maps(x, w_qkv, w_proj, b_proj, w_dwc, b_dwc):
    x = np.asarray(x, dtype=np.float32)
    N = x.shape[0]
    assert N == N_CORES * S
    import ml_dtypes
    wqkvT = np.ascontiguousarray(
        np.asarray(w_qkv, np.float32).T.astype(ml_dtypes.bfloat16))    # [C, 3C]
    wprojT = np.ascontiguousarray(
        np.asarray(w_proj, np.float32).T.astype(ml_dtypes.bfloat16))   # [C, C]
    wdwc9 = np.ascontiguousarray(np.asarray(w_dwc, np.float32).reshape(C, 9))
    bdwc = np.ascontiguousarray(np.asarray(b_dwc, np.float32).reshape(C, 1))
    bproj = np.ascontiguousarray(np.asarray(b_proj, np.float32).reshape(1, C))

    in_maps = []
    for i in range(N_CORES):
        xs = x[i * S:(i + 1) * S]                       # [S, L, C]
        xT = np.ascontiguousarray(
            xs.transpose(0, 2, 1).astype(ml_dtypes.bfloat16))  # [S, C, L]
        in_maps.append({"xT": xT, "wqkvT": wqkvT, "wprojT": wprojT,
                        "wdwc": wdwc9, "bdwc": bdwc, "bproj": bproj})
    return in_maps


def kernel(x, w_qkv, w_proj, b_proj, w_dwc, b_dwc):
    global last_results
    nc = _build_nc()
    in_maps = make_in_maps(x, w_qkv, w_proj, b_proj, w_dwc, b_dwc)
    last_results = run_bass_kernel_spmd(nc, in_maps, list(range(N_CORES)))
    y = np.concatenate([r["y"] for r in last_results.results], axis=0)
    return y.astype(np.float32)


# revision 27
# speedup vs baseline: 1.0210x; 1.0151x over previous
"""Trainium2 Bass kernel for DeiT-style attention + depthwise-conv block.

Computes, for x [N=32, L=577, C=768]:
  qkv = x @ w_qkv.T -> q,k,v (12 heads, hd=64)
  attn = softmax(q k^T / 8) @ v
  out  = attn (+ depthwise3x3(v) on patch tokens) @ w_proj.T + b_proj

Sharding: data-parallel over batch, 4 samples per core x 8 NeuronCores.

Structure (per core): the attention inner loop is ACT(exp)-paced, so PE
work from the NEXT sample (qkv/v65 matmuls) and the TWO-samples-back
projection is interleaved as filler between each head's scores and PV
matmuls.  Head pairs write scores into ONE shared [128, 2L] PSUM tile
(disjoint PE row groups run concurrently; disjoint PSUM banks) so exp
runs as a single ACT op per chunk-pair, halving ACT instruction count.
Weights are SBUF-resident.  Softmax normalize = ACT sums-copy -> DVE
fast reciprocal -> gpsimd partition-broadcast -> DVE multiply
(reciprocal_approx_fast requires a base-partition-0 SBUF input on
hardware).  The depthwise conv runs on the DVE as 9 fused
scalar_tensor_tensor taps over flat dense windows of two zero-bordered
pad buffers (the second shifted by one element so odd-offset taps stay
4B-aligned for the 2x DVE mode).
"""
import sys

sys.path.insert(0, "/opt/trn_rl_repo")

import numpy as np

import concourse.bacc as bacc
import concourse.mybir as mybir
import concourse.tile as tile
from concourse.bass_utils import run_bass_kernel_spmd

F32 = mybir.dt.float32
F32R = mybir.dt.float32r
BF16 = mybir.dt.bfloat16
Exp = mybir.ActivationFunctionType.Exp
MULT = mybir.AluOpType.mult
ADD = mybir.AluOpType.add
DIV = mybir.AluOpType.divide

N_CORES = 8
S = 4            # samples per core
C, L, H, HD = 768, 577, 12, 64
CT = C // 128    # 6 channel tiles
KT = 3 * C // 128  # 18 qkv row tiles
SCALE = HD ** -0.5
L_CHUNKS = [(i * 128, min(128, L - i * 128)) for i in range((L + 127) // 128)]
NCH = len(L_CHUNKS)  # 5
IMG = 24         # spatial side; L-1 == IMG*IMG
PAD = IMG + 2    # padded side
PP = PAD * PAD   # 676
CW = PP - 2 * PAD - 2  # 622: flat conv window length
# tap offsets into the flat padded buffer: delta = PAD*dy + dx
EVEN_TAPS = [(0, 0), (2, 2), (26, 3), (28, 5), (52, 6), (54, 8)]  # (delta, w-idx)
ODD_TAPS = [(1, 1), (27, 4), (53, 7)]  # read from the 1-shifted buffer at delta-1

CONV_PE = False  # True: depthwise conv via diag-matmuls on TensorE

# eviction split knobs: 1-in-N PSUM->SBUF evictions go to ACT
EV_ACT_FILL = 2
EV_ACT_ATTN = 10 ** 9

_CACHE = {}
last_results = None  # BassKernelResults of the most recent run (for test harness)

def _build_nc(repeat=1, stages="full"):
    key = (repeat, stages)
    if key in _CACHE:
        return _CACHE[key]
    nc = bacc.Bacc("TRN2", target_bir_lowering=False, debug=False,
                   num_devices=N_CORES)
    xT_d = nc.declare_dram_parameter("xT", [S, C, L], BF16, isOutput=False)
    wqkvT_d = nc.declare_dram_parameter("wqkvT", [C, 3 * C], BF16, isOutput=False)
    wprojT_d = nc.declare_dram_parameter("wprojT", [C, C], BF16, isOutput=False)
    wdwc_d = nc.declare_dram_parameter("wdwc", [C, 9], F32, isOutput=False)
    bdwc_d = nc.declare_dram_parameter("bdwc", [C, 1], F32, isOutput=False)
    bproj_d = nc.declare_dram_parameter("bproj", [1, C], F32, isOutput=False)
    y_d = nc.declare_dram_parameter("y", [S, L, C], F32, isOutput=True)

    with tile.TileContext(nc) as tc:
        with tc.tile_pool(name="wpool", bufs=1) as wpool, \
             tc.tile_pool(name="work", bufs=2) as work, \
             tc.tile_pool(name="mm", bufs=1, space="PSUM") as psum_mm, \
             tc.tile_pool(name="sc", bufs=2, space="PSUM") as psum_sc, \
             tc.tile_pool(name="pv", bufs=1, space="PSUM") as psum_pv:

            # ---- resident weights (loaded once; q parts first so the
            # first qkv matmuls can start before the rest arrives) ----
            wqkv = []
            for k in range(CT):
                t = wpool.tile([128, 3 * C], BF16, tag="wqkv", bufs=CT,
                               name=f"wqkv{k}")
                wqkv.append(t)
            # single-shot build: prefetch sample-0 x ahead of the weight
            # loads so the first qkv matmuls can start as early as possible
            pre_x0 = None
            if repeat == 1:
                pre_x0 = []
                for k in range(CT):
                    t = work.tile([128, L], BF16, tag="xT", bufs=2 * CT,
                                  name=f"xT0p{k}")
                    nc.sync.dma_start(t[:], xT_d[0, k * 128:(k + 1) * 128, :])
                    pre_x0.append(t)

            wdma = nc.sync
            for part in range(3):
                for k in range(CT):
                    wdma.dma_start(
                        wqkv[k][:, part * C:(part + 1) * C],
                        wqkvT_d[k * 128:(k + 1) * 128, part * C:(part + 1) * C])
            wprojT = []
            for k in range(CT):
                t = wpool.tile([128, C], BF16, tag="wprojT", bufs=CT,
                               name=f"wprojT{k}")
                wdma.dma_start(t[:], wprojT_d[k * 128:(k + 1) * 128, :])
                wprojT.append(t)
            wdwc = []
            bdwc = []
            for k in range(CT):
                t = wpool.tile([128, 9], F32, tag="wdwc", bufs=CT, name=f"wdwc{k}")
                wdma.dma_start(t[:], wdwc_d[k * 128:(k + 1) * 128, :])
                wdwc.append(t)
                t = wpool.tile([128, 1], F32, tag="bdwc", bufs=CT, name=f"bdwc{k}")
                wdma.dma_start(t[:], bdwc_d[k * 128:(k + 1) * 128, :])
                bdwc.append(t)
            bproj_row = wpool.tile([1, C], F32, tag="bprow")
            wdma.dma_start(bproj_row[:], bproj_d[:])
            bproj_bc = wpool.tile([128, C], F32, tag="bpbc")
            nc.gpsimd.partition_broadcast(bproj_bc[:], bproj_row[:])

            # persistent zero-bordered conv pad buffers (interior rewritten
            # per use; borders stay zero).  vpadB holds the same image
            # shifted left by one flat element so odd tap offsets become
            # even (4B-aligned) reads.
            vpad = []
            for i in range(2):
                t = wpool.tile([128, PP], BF16, tag="vpad", bufs=2,
                               name=f"vpad{i}")
                nc.vector.memset(t[:], 0.0)
                vpad.append(t)
            # persistent v65 tiles (two sets of NCH); ones column written once
            v65_all = []
            for i in range(2 * NCH):
                t = wpool.tile([128, H * 65], BF16, tag="v65", bufs=2 * NCH,
                               name=f"v65_{i}")
                t3 = t[:].rearrange("p (h w) -> p h w", h=H, w=65)
                nc.vector.memset(t3[:, :, 64:65], 1.0)
                v65_all.append(t)

            import contextlib
            rep_ctx = tc.For_i(0, repeat, 1) if repeat > 1 else contextlib.nullcontext()
            with rep_ctx:
                state = {}
                evict_ctr = [0]
                evict_act_period = [EV_ACT_FILL]  # 1-in-N evictions go to ACT

                def evict(dst_ap, src_ap):
                    # split PSUM->SBUF evictions between ACT and DVE
                    if evict_ctr[0] % evict_act_period[0] == 0:
                        nc.scalar.copy(dst_ap, src_ap)
                    else:
                        nc.vector.tensor_copy(dst_ap, src_ap)
                    evict_ctr[0] += 1

                mm_ctr = [0]
                mm_alt = [False]  # when True, alternate mm/sc pools

                def mm_tile():
                    mm_ctr[0] += 1
                    if mm_alt[0] and mm_ctr[0] % 2 == 0:
                        return psum_sc.tile([128, 768], F32, tag="sc",
                                            name="mmsc")
                    return psum_mm.tile([128, 768], F32, tag="mm", name="mmp")

                def emit_sample_inputs(s):
                    st = {"xT": [], "qk": [], "vch": [],
                          "v65": [v65_all[(s % 2) * NCH + ci] for ci in range(NCH)]}
                    if s == 0 and pre_x0 is not None:
                        st["xT"] = pre_x0
                        state[s] = st
                        return st
                    for k in range(CT):
                        t = work.tile([128, L], BF16, tag="xT", bufs=2 * CT,
                                      name=f"xT{k}")
                        nc.sync.dma_start(t[:], xT_d[s, k * 128:(k + 1) * 128, :])
                        st["xT"].append(t)
                    state[s] = st
                    return st

                def qkv_mtile(s, m):
                    st = state[s]
                    p = mm_tile()
                    for k in range(CT):
                        w_ap = wqkv[k][:, m * 128:(m + 1) * 128]
                        for (n0, nn) in ((0, 512), (512, 65)):
                            nc.tensor.matmul(
                                p[:, n0:n0 + nn], w_ap,
                                st["xT"][k][:, n0:n0 + nn],
                                start=(k == 0), stop=(k == CT - 1))
                    if m < 12:
                        dst = work.tile([128, L], BF16, tag="qk", bufs=24,
                                        name=f"qkv{m}")
                        evict(dst[:], p[:, 0:L])
                        st["qk"].append(dst)
                    else:
                        # vch padded to 640 so the tail v65 transpose can
                        # read a full 128-wide window (xbar constraint)
                        dst = work.tile([128, 640], BF16, tag="vch",
                                        bufs=2 * CT, name=f"qkv{m}")
                        evict(dst[:, 0:L], p[:, 0:L])
                        nc.vector.memset(dst[:, L:640], 0.0)
                        st["vch"].append(dst)

                def v65_chunk(s, ci):
                    # v65 [l, c] from vch [c, l] by xbar DMA transpose into a
                    # 128B-aligned temp (misaligned 65-stride dests corrupt
                    # on hardware), then one DVE copy into the [v|1] layout;
                    # frees the TensorE of the duplicate v matmuls.  The tail
                    # chunk reads the zero-padded 512:640 window; its junk
                    # rows (>= lp) are never read by PV.
                    st = state[s]
                    (l0, lp) = L_CHUNKS[ci]
                    w0 = min(l0, 640 - 128)
                    t = st["v65"][ci]
                    tmp = work.tile([128, 768], BF16, tag="v65t", bufs=2,
                                    name="v65t")
                    for cj in range(CT):
                        # all transposes on the SP HWDGE queue: the ACT queue
                        # is strict-FIFO depth-8, so a transpose waiting on
                        # its vch input at the queue head would stall the exp
                        # stream behind it (ACT paces the attention loop)
                        eng = nc.sync
                        eng.dma_start_transpose(
                            tmp[0:128, cj * 128:(cj + 1) * 128],
                            st["vch"][cj][0:128, w0:w0 + 128])
                    nc.vector.tensor_copy(
                        t[0:lp].rearrange("p (h w) -> p h w", h=H, w=65)[:, :, 0:64],
                        tmp[0:lp].rearrange("p (h w) -> p h w", h=H, w=64))

                def make_fill_thunks(s):
                    # vch m-tiles (12..17) first so the v65 transposes can
                    # start as early as possible
                    return ([lambda m=m: qkv_mtile(s, m)
                             for m in list(range(12, KT)) + list(range(12))]
                            + [lambda ci=ci: v65_chunk(s, ci) for ci in range(NCH)])

                def proj_chunk(s, ci):
                    st = state[s]
                    (l0, lp) = L_CHUNKS[ci]
                    attn = st["attn"]
                    p = mm_tile()
                    for (n0, nn) in ((0, 512), (512, 256)):
                        for k in range(CT):
                            nc.tensor.matmul(
                                p[0:lp, n0:n0 + nn],
                                attn[k][:, l0:l0 + lp],
                                wprojT[k][:, n0:n0 + nn],
                                start=(k == 0), stop=(k == CT - 1))
                    ysb = work.tile([128, C], F32, tag="ysb", bufs=2)
                    nc.vector.tensor_tensor(
                        out=ysb[0:lp, :], in0=p[0:lp, :], in1=bproj_bc[0:lp, :],
                        op=ADD)
                    nc.sync.dma_start(y_d[s, l0:l0 + lp, :], ysb[0:lp, :])

                def make_proj_thunks(s):
                    return [lambda ci=ci: proj_chunk(s, ci) for ci in range(NCH)]

                def scores_pair(s, hp):
                    # heads 2hp (rows 0:64) and 2hp+1 (rows 64:128) emitted
                    # chunk-interleaved: adjacent matmuls hit disjoint PE row
                    # groups AND disjoint PSUM banks (separate sc-ring slots),
                    # so they run concurrently on hardware
                    st = state[s]
                    qt = st["qk"][hp]
                    kt_ = st["qk"][6 + hp]
                    expA, expB = [], []
                    order = [(l0, lp, hb, e) for (l0, lp) in L_CHUNKS
                             for hb, e in ((0, expA), (64, expB))]
                    for (l0, lp, hb, exps) in order:
                        p = psum_sc.tile([128, 768], F32, tag="sc",
                                         name="scp")
                        for (n0, nn) in ((0, 512), (512, 65)):
                            nc.tensor.matmul(p[0:lp, n0:n0 + nn],
                                             kt_[hb:hb + 64, l0:l0 + lp],
                                             qt[hb:hb + 64, n0:n0 + nn],
                                             start=True, stop=True)
                        e = work.tile([128, L], BF16, tag="expS", bufs=12,
                                      name="expSt")
                        nc.scalar.activation(e[0:lp, :], p[0:lp, 0:L], Exp,
                                             scale=SCALE)
                        exps.append(e)
                    return expA, expB

                def pv_head(s, h, exps):
                    st = state[s]
                    pv = psum_pv.tile([128, L], F32, tag="pv")
                    for ci, (l0, lp) in enumerate(L_CHUNKS):
                        for (n0, nn) in ((0, 512), (512, 65)):
                            nc.tensor.matmul(
                                pv[0:65, n0:n0 + nn],
                                st["v65"][ci][0:lp, h * 65:(h + 1) * 65],
                                exps[ci][0:lp, n0:n0 + nn],
                                start=(ci == 0), stop=(ci == NCH - 1))
                    # NOTE: reciprocal_approx_fast needs a base-partition-0
                    # SBUF operand (PSUM or partition-offset reads return
                    # garbage on hardware), so stage the sums row via ACT
                    sums = work.tile([1, L], F32, tag="sums", bufs=2,
                                     name="sums")
                    nc.scalar.copy(sums[:], pv[64:65, :])
                    rec = work.tile([1, L], F32, tag="rec", bufs=2, name="rec")
                    nc.vector.reciprocal_approx_fast(out=rec[:], in_=sums[:])
                    bc = work.tile([64, L], F32, tag="bc", bufs=2, name="bc")
                    nc.gpsimd.partition_broadcast(bc[:], rec[:])
                    hb = (h % 2) * 64
                    nc.vector.tensor_tensor(
                        out=st["attn"][h // 2][hb:hb + 64, :],
                        in0=pv[0:64, :], in1=bc[:], op=MULT)

                def conv_prep(s, ct):
                    st = state[s]
                    vp = vpad[ct % 2]
                    vp3 = vp[:].rearrange("p (y x) -> p y x", y=PAD, x=PAD)
                    src_ = st["vch"][ct][:, 1:L].rearrange("p (y x) -> p y x",
                                                           y=IMG, x=IMG)
                    nc.vector.tensor_copy(vp3[:, 1:1 + IMG, 1:1 + IMG], src_)
                    if CONV_PE:
                        vpB = vpadB[ct % 2]
                        vpB3 = vpB[:].rearrange("p (y x) -> p y x",
                                                y=PAD, x=PAD)
                        nc.vector.tensor_copy(vpB3[:, 1:1 + IMG, 0:IMG], src_)
                        p = psum_sc.tile([128, 768], F32, tag="sc",
                                         name="caccp")
                        taps = ([(d, w, vp) for (d, w) in EVEN_TAPS]
                                + [(d - 1, w, vpB) for (d, w) in ODD_TAPS])
                        for (n0, nn) in ((0, 512), (512, 110)):
                            for i, (dd, w, buf) in enumerate(taps):
                                nc.tensor.matmul(
                                    p[:, n0:n0 + nn],
                                    wdiag[ct][:, w * 128:(w + 1) * 128],
                                    buf[:, dd + n0:dd + n0 + nn],
                                    start=(i == 0), stop=(i == 8))
                        return p
                    acc = work.tile([128, IMG * IMG], BF16, tag="cacc", bufs=2,
                                    name="cacc")
                    acc3 = acc[:].rearrange("p (y x) -> p y x", y=IMG, x=IMG)

                    def tap(t):
                        return vp3[:, t // 3:t // 3 + IMG, t % 3:t % 3 + IMG]

                    nc.vector.tensor_scalar(
                        out=acc3, in0=tap(4), scalar1=wdwc[ct][:, 4:5],
                        scalar2=None, op0=MULT)
                    for t in [0, 1, 2, 3, 5, 6, 7, 8]:
                        tmp = work.tile([128, IMG * IMG], BF16, tag="ctmp",
                                        bufs=6, name="ctmp")
                        tmp3 = tmp[:].rearrange("p (y x) -> p y x", y=IMG, x=IMG)
                        nc.vector.tensor_scalar(
                            out=tmp3, in0=tap(t), scalar1=wdwc[ct][:, t:t + 1],
                            scalar2=None, op0=MULT)
                        nc.vector.tensor_tensor(out=acc[:], in0=acc[:],
                                                in1=tmp[:], op=ADD)
                    return acc

                def conv_add(s, ct, acc):
                    if CONV_PE:
                        acc_int = acc[:, 0:IMG * PAD].rearrange(
                            "p (y x) -> p y x", y=IMG, x=PAD)[:, :, 0:IMG]
                        nc.vector.scalar_tensor_tensor(
                            out=state[s]["attn"][ct][:, 1:L].rearrange(
                                "p (y x) -> p y x", y=IMG, x=IMG),
                            in0=acc_int,
                            scalar=bdwc[ct][:, 0:1],
                            in1=state[s]["attn"][ct][:, 1:L].rearrange(
                                "p (y x) -> p y x", y=IMG, x=IMG),
                            op0=ADD, op1=ADD)
                        return
                    # attn[:, 1:] += acc + b_dwc
                    nc.vector.scalar_tensor_tensor(
                        out=state[s]["attn"][ct][:, 1:L], in0=acc[:],
                        scalar=bdwc[ct][:, 0:1],
                        in1=state[s]["attn"][ct][:, 1:L],
                        op0=ADD, op1=ADD)

                # ---- prologue: sample 0 inputs + qkv/v65 emitted directly
                # (mm/sc pool alternation -- nothing else needs sc yet) ----
                emit_sample_inputs(0)
                mm_alt[0] = True
                for t in make_fill_thunks(0):
                    t()
                mm_alt[0] = False

                for s in range(S):
                    st = state[s]
                    st["attn"] = [work.tile([128, L], BF16, tag="attn", bufs=18,
                                            name=f"attn{ct}") for ct in range(CT)]
                    fillers = []
                    if s + 1 < S:
                        emit_sample_inputs(s + 1)
                        fillers += make_fill_thunks(s + 1)
                    # projections trail by two samples so the tail sample's
                    # exp-waits still have PE filler work
                    if s - 2 >= 0:
                        fillers += make_proj_thunks(s - 2)
                    if s == S - 1:
                        fillers += make_proj_thunks(s - 1)

                    if stages == "qkv":
                        zsrc = work.tile([128, L], F32, tag="zsrc", bufs=1,
                                         name="zsrc")
                        nc.vector.memset(zsrc[:], 0.0)
                        for ct in range(CT):
                            nc.vector.tensor_copy(st["attn"][ct][:], zsrc[:])
                        for t in fillers:
                            t()
                        continue

                    # ACT paces the attention inner loop; keep it mostly exp
                    evict_act_period[0] = EV_ACT_ATTN
                    nf = len(fillers)
                    done = 0
                    for hp in range(H // 2):
                        expA, expB = scores_pair(s, hp)
                        if stages == "full":
                            acc = conv_prep(s, hp)
                        # PE fillers between scores and PV cover the exp wait
                        target = ((2 * hp + 1) * nf) // H
                        while done < target:
                            fillers[done]()
                            done += 1
                        pv_head(s, 2 * hp, expA)
                        target = ((2 * hp + 2) * nf) // H
                        while done < target:
                            fillers[done]()
                            done += 1
                        pv_head(s, 2 * hp + 1, expB)
                        if stages == "full":
                            conv_add(s, hp, acc)
                    evict_act_period[0] = EV_ACT_FILL

                # final projection (no attention loop left to hide it in)
                mm_alt[0] = True
                for t in make_proj_thunks(S - 1):
                    t()
                mm_alt[0] = False

    nc.compile()
    _CACHE[key] = nc
    return nc


def make_in_maps(x, w_qkv, w_proj, b_proj, w_dwc, b_dwc):
    x = np.asarray(x, dtype=np.float32)
    N = x.shape[0]
    assert N == N_CORES * S
    import ml_dtypes
    wqkvT = np.ascontiguousarray(
        np.asarray(w_qkv, np.float32).T.astype(ml_dtypes.bfloat16))    # [C, 3C]
    wprojT = np.ascontiguousarray(
        np.asarray(w_proj, np.float32).T.astype(ml_dtypes.bfloat16))   # [C, C]
    wdwc9 = np.ascontiguousarray(np.asarray(w_dwc, np.float32).reshape(C, 9))
    bdwc = np.ascontiguousarray(np.asarray(b_dwc, np.float32).reshape(C, 1))
    bproj = np.ascontiguousarray(np.asarray(b_proj, np.float32).reshape(1, C))

    in_maps = []
    for i in range(N_CORES):
        xs = x[i * S:(i + 1) * S]                       # [S, L, C]
        xT = np.ascontiguousarray(
            xs.transpose(0, 2, 1).astype(ml_dtypes.bfloat16))  # [S, C, L]
        in_maps.append({"xT": xT, "wqkvT": wqkvT, "wprojT": wprojT,
                        "wdwc": wdwc9, "bdwc": bdwc, "bproj": bproj})
    return in_maps


def kernel(x, w_qkv, w_proj, b_proj, w_dwc, b_dwc):
    global last_results
    nc = _build_nc()
    in_maps = make_in_maps(x, w_qkv, w_proj, b_proj, w_dwc, b_dwc)
    last_results = run_bass_kernel_spmd(nc, in_maps, list(range(N_CORES)))
    y = np.concatenate([r["y"] for r in last_results.results], axis=0)
    return y.astype(np.float32)
